# revision 9
# baseline (speedup 1.0000x reference)
"""GCN (4x GCNConv + 1 output GCNConv) on 8 Trainium2 NeuronCores.

Strategy (node-sharded, replicated scaled-feature table):
  h_{l+1} = relu(A h_l W_l + b_l),  A = D^-1/2 (Adj+I) D^-1/2

  Let ht = dinv * h (pre-scaled rows). Then
    (A h)_v = dinv_v * sum_{e: dst=v} ht[src_e]
  so all edge messages are plain gathered rows of ht (edge weights fold away).

  Per core (owns 12500 dst nodes):
    - full replica of ht lives in core-local DRAM, in "quarter-permuted" row
      order so a 4-way-split AllGather writes it directly, and every source
      block of 25000 rows is int16-indexable for dma_gather.
    - edges (dst in shard) are bucketed by (table block b, dst tile t of 128),
      padded to 128-edge chunks on a schedule shared by all 8 cores
      (SPMD: one instruction stream, per-core data).
    - per chunk: dma_gather 128 rows -> M [128e, F]; DVE is_equal builds the
      0/1 selection S [128e, 128dst] from dst-local offsets vs a resident
      iota; PE matmul accumulates aggT[F, 128dst] += M^T-contract-S in PSUM.
    - block-partial aggregates are accumulated in SBUF so compute on block k
      only depends on AllGather k (overlap).
    - eviction per tile: aggT @ W in PE, then out = relu(dinv*(dinv*u)+b)
      fused as 2 DVE ops + 1 ACT op (per-partition scale), DMA to shard.
    - 4 AllGathers ship the new shard quarters into the next layer's table.
"""

import numpy as np

# ---------------------------------------------------------------- constants
N = 100000
E = 1600000
F_IN = 64
H = 128
L = 3  # hidden GCN layers with W2
NCORES = 8
NPC = N // NCORES        # 12500 nodes per core
DTILE = 128              # dst nodes per output tile
TPC = (NPC + DTILE - 1) // DTILE   # 98 dst tiles per core
NBLK = 4                 # int16 source blocks
BLK = N // NBLK          # 25000 table rows per block
Q = NPC // NBLK          # 3125 shard rows per AllGather quarter
GROUP = 5                # dst tiles per gather group
NGRP = (TPC + GROUP - 1) // GROUP  # 20
CHUNK = 128

_CACHE = {}


# ---------------------------------------------------------------- host prep
def _preprocess(edge_index):
    ei = np.asarray(edge_index, dtype=np.int64)
    src = np.concatenate([ei[0], np.arange(N, dtype=np.int64)])
    dst = np.concatenate([ei[1], np.arange(N, dtype=np.int64)])
    deg = np.bincount(dst, minlength=N).astype(np.float32)
    dinv = np.where(deg > 0, 1.0 / np.sqrt(deg), 0.0).astype(np.float32)

    # node s -> table row r (quarter-permuted so AllGather k writes block k)
    s_all = np.arange(N, dtype=np.int64)
    c_of, j_of = s_all // NPC, s_all % NPC
    k_of = j_of // Q
    r_of = k_of * BLK + c_of * Q + (j_of % Q)

    r_src = r_of[src]
    b_e = r_src // BLK
    iloc = (r_src % BLK).astype(np.int64)
    core_e = dst // NPC
    dl = dst % NPC
    t_e = dl // DTILE
    dloc = (dl % DTILE).astype(np.float32)

    key = (core_e * TPC + t_e) * NBLK + b_e
    cnt = np.bincount(key, minlength=NCORES * TPC * NBLK).reshape(NCORES, TPC, NBLK)
    cpt = np.ceil(cnt.max(axis=0) / CHUNK).astype(np.int64)  # [TPC, NBLK]

    # schedule: chunks laid out in (b, g, t, j) order
    chunk_base = np.zeros((TPC, NBLK), np.int64)
    ncol_bg = np.zeros((NBLK, NGRP), np.int64)
    choff_bg = np.zeros((NBLK, NGRP), np.int64)
    cb = 0
    for b in range(NBLK):
        for g in range(NGRP):
            choff_bg[b, g] = cb
            for t in range(g * GROUP, min((g + 1) * GROUP, TPC)):
                chunk_base[t, b] = cb
                cb += cpt[t, b]
            ncol_bg[b, g] = cb - choff_bg[b, g]
    NCH = int(cb)
    NIT = NCH * CHUNK

    # per-edge slot position: chunk_base[t,b]*128 + rank within (core,t,b)
    ordk = np.argsort(key, kind="stable")
    ks = key[ordk]
    changes = np.nonzero(np.diff(ks))[0] + 1
    seg_starts = np.concatenate([[0], changes])
    seg_lens = np.diff(np.concatenate([seg_starts, [len(ks)]]))
    rank_sorted = np.arange(len(ks)) - np.repeat(seg_starts, seg_lens)
    rank = np.empty(len(ks), np.int64)
    rank[ordk] = rank_sorted
    pos = chunk_base[t_e, b_e] * CHUNK + rank

    idx_blobs, dst_blobs, dinv_tiles = [], [], []
    for c in range(NCORES):
        sel = core_e == c
        ib = np.zeros(NIT, np.int64)
        db = np.full(NIT, -1.0, np.float32)
        ib[pos[sel]] = iloc[sel]
        db[pos[sel]] = dloc[sel]
        idx_blobs.append(np.tile(ib.reshape(NIT // 16, 16).T.astype(np.int16), (8, 1)))
        dst_blobs.append(db.reshape(NCH, CHUNK).T.copy())  # [128, NCH]
        dv = np.zeros(TPC * DTILE, np.float32)
        dv[:NPC] = dinv[c * NPC:(c + 1) * NPC]
        dinv_tiles.append(dv.reshape(TPC, DTILE).T.copy())  # [128, TPC]

    first_b = np.full(TPC, -1, np.int64)
    last_b = np.full(TPC, -1, np.int64)
    for t in range(TPC):
        nz = np.nonzero(cpt[t])[0]
        if len(nz):
            first_b[t] = nz[0]
            last_b[t] = nz[-1]

    sched = dict(cpt=cpt, ncol_bg=ncol_bg, choff_bg=choff_bg,
                 chunk_base=chunk_base, first_b=first_b, last_b=last_b,
                 NCH=NCH, NIT=NIT)
    return sched, dinv, r_of, idx_blobs, dst_blobs, dinv_tiles


# ---------------------------------------------------------------- bass build
def _build(sched):
    from concourse import bacc, mybir
    from concourse.tile import TileContext

    f32, i16 = mybir.dt.float32, mybir.dt.int16
    nc = bacc.Bacc("TRN2", target_bir_lowering=False, debug=False,
                   num_devices=NCORES, num_swdge_queues=4)

    NIT, NCH = sched["NIT"], sched["NCH"]
    cpt, ncol_bg, choff_bg = sched["cpt"], sched["ncol_bg"], sched["choff_bg"]
    first_b, last_b = sched["first_b"], sched["last_b"]

    xt = nc.dram_tensor("xt", [N, F_IN], f32, kind="ExternalInput")
    idx_in = nc.dram_tensor("idx16", [128, NIT // 16], i16, kind="ExternalInput")
    dstl_in = nc.dram_tensor("dstloc", [128, NCH], f32, kind="ExternalInput")
    iota_in = nc.dram_tensor("iota", [128, DTILE], f32, kind="ExternalInput")
    dinv_in = nc.dram_tensor("dinvt", [128, TPC], f32, kind="ExternalInput")
    w_in, b_in = [], []
    fouts = [H] + [H] * L + [1]
    fins = [F_IN] + [H] * L + [H]
    for l in range(L + 2):
        w_in.append(nc.dram_tensor(f"w{l}", [fins[l], fouts[l]], f32,
                                   kind="ExternalInput"))
        b_in.append(nc.dram_tensor(f"bias{l}", [128, fouts[l]], f32,
                                   kind="ExternalInput"))
    out_d = nc.dram_tensor("out", [NPC, 1], f32, kind="ExternalOutput")

    tabs = [None]
    for l in range(1, L + 2):
        tabs.append(nc.dram_tensor(f"tab{l}", [N, H], f32, kind="Internal",
                                   addr_space="Shared"))
    shards = []
    for l in range(L + 1):
        shards.append(nc.dram_tensor(f"shard{l}", [TPC * DTILE, H], f32,
                                     kind="Internal"))

    with TileContext(nc) as tc:
        with tc.tile_pool(name="res", bufs=1) as res, \
             tc.tile_pool(name="gb", bufs=3) as gpool, \
             tc.tile_pool(name="sb", bufs=3) as spool, \
             tc.tile_pool(name="acc", bufs=TPC + 2) as accpool, \
             tc.tile_pool(name="ev", bufs=4) as evpool, \
             tc.tile_pool(name="ps1", bufs=5, space="PSUM") as ps1pool, \
             tc.tile_pool(name="ps2", bufs=3, space="PSUM") as ps2pool:

            idx_t = res.tile([128, NIT // 16], i16)
            nc.sync.dma_start(out=idx_t[:], in_=idx_in[:])
            dstl_t = res.tile([128, NCH], f32)
            nc.sync.dma_start(out=dstl_t[:], in_=dstl_in[:])
            iota_t = res.tile([128, DTILE], f32)
            nc.sync.dma_start(out=iota_t[:], in_=iota_in[:])
            dinv_t = res.tile([128, TPC], f32)
            nc.sync.dma_start(out=dinv_t[:], in_=dinv_in[:])
            w_t, bias_t = [], []
            for l in range(L + 2):
                wt = res.tile([fins[l], fouts[l]], f32, tag=f"w{l}")
                nc.sync.dma_start(out=wt[:], in_=w_in[l][:])
                bt = res.tile([128, fouts[l]], f32, tag=f"b{l}")
                nc.sync.dma_start(out=bt[:], in_=b_in[l][:])
                w_t.append(wt)
                bias_t.append(bt)

            qctr = [0]

            def run_layer(l, table, out_shard, out_final):
                fi, fo = fins[l], fouts[l]
                acc = {}

                def evict(t):
                    # aggT complete for tile t: apply W, dinv/bias/relu, store.
                    # Inlined at the tile's last block so it overlaps the rest
                    # of the b-loop and unblocks the next layer's AllGathers.
                    ps2 = ps2pool.tile([DTILE, fo], f32, tag="ps2")
                    nc.tensor.matmul(out=ps2[:], lhsT=acc[t][:], rhs=w_t[l][:],
                                     start=True, stop=True)
                    nc.vector.tensor_scalar(
                        out=ps2[:], in0=ps2[:], scalar1=dinv_t[:, t:t + 1],
                        scalar2=None, op0=mybir.AluOpType.mult)
                    nc.vector.tensor_tensor(
                        out=ps2[:], in0=ps2[:], in1=bias_t[l][:],
                        op=mybir.AluOpType.add)
                    ev = evpool.tile([DTILE, fo], f32, tag="ev")
                    if out_final is not None:
                        nc.scalar.activation(
                            out=ev[:], in_=ps2[:],
                            func=mybir.ActivationFunctionType.Copy)
                    else:
                        nc.scalar.activation(
                            out=ev[:], in_=ps2[:],
                            func=mybir.ActivationFunctionType.Relu,
                            scale=dinv_t[:, t:t + 1])
                    rows = min(DTILE, NPC - t * DTILE)
                    dst = out_final if out_final is not None else out_shard
                    nc.sync.dma_start(
                        out=dst[t * DTILE:t * DTILE + rows, :],
                        in_=ev[:rows, :])

                for b in range(NBLK):
                    for g in range(NGRP):
                        ncol = int(ncol_bg[b, g])
                        if ncol == 0:
                            continue
                        ch0 = int(choff_bg[b, g])
                        gt = gpool.tile([128, ncol, fi], f32, tag="g")
                        # SWDGE descriptor ring caps one gather at ~1024 idx;
                        # split into <=8-chunk calls across the 4 queues.
                        for c0 in range(0, ncol, 8):
                            cw = min(8, ncol - c0)
                            nc.gpsimd.dma_gather(
                                out_ap=gt[:, c0:c0 + cw, :],
                                in_ap=table[b * BLK:(b + 1) * BLK, :],
                                idxs_ap=idx_t[:, (ch0 + c0) * 8:
                                              (ch0 + c0 + cw) * 8],
                                num_idxs=cw * CHUNK, num_idxs_reg=cw * CHUNK,
                                elem_size=fi,
                                queue_num=qctr[0] % 4)
                            qctr[0] += 1
                        st = spool.tile([128, ncol, DTILE], f32, tag="s")
                        nc.vector.tensor_tensor(
                            out=st[:],
                            in0=dstl_t[:, ch0:ch0 + ncol].unsqueeze(-1)
                                .to_broadcast([128, ncol, DTILE]),
                            in1=iota_t[:].unsqueeze(1)
                                .to_broadcast([128, ncol, DTILE]),
                            op=mybir.AluOpType.is_equal)
                        col = 0
                        for t in range(g * GROUP, min((g + 1) * GROUP, TPC)):
                            cp = int(cpt[t, b])
                            if cp == 0:
                                continue
                            ps = ps1pool.tile([fi, DTILE], f32, tag="ps1")
                            for j in range(cp):
                                nc.tensor.matmul(
                                    out=ps[:], lhsT=gt[:, col, :],
                                    rhs=st[:, col, :],
                                    start=(j == 0), stop=(j == cp - 1))
                                col += 1
                            if first_b[t] == b:
                                at = accpool.tile([fi, DTILE], f32, tag="acc")
                                acc[t] = at
                                nc.any.tensor_copy(out=at[:], in_=ps[:])
                            else:
                                nc.vector.tensor_tensor(
                                    out=acc[t][:], in0=acc[t][:], in1=ps[:],
                                    op=mybir.AluOpType.add)
                            if last_b[t] == b:
                                evict(t)

            for l in range(L + 2):
                last = l == L + 1
                table = xt[:] if l == 0 else tabs[l][:]
                run_layer(l, table,
                          None if last else shards[l][:],
                          out_d[:] if last else None)
                if not last:
                    for k in range(NBLK):
                        nc.gpsimd.collective_compute(
                            "AllGather", mybir.AluOpType.bypass,
                            replica_groups=[list(range(NCORES))],
                            ins=[shards[l][k * Q:(k + 1) * Q, :]],
                            outs=[tabs[l + 1][k * BLK:(k + 1) * BLK, :]])

    nc.finalize()
    return nc


# ---------------------------------------------------------------- entry
def kernel(x, edge_index, W1, b1, W2, b2, W3, b3):
    import sys
    if "/opt/trn_rl_repo" not in sys.path:
        sys.path.insert(0, "/opt/trn_rl_repo")
    from concourse import bass_utils

    x = np.asarray(x, dtype=np.float32)
    edge_index = np.asarray(edge_index)

    import hashlib
    key = hashlib.sha1(np.ascontiguousarray(edge_index)).hexdigest()
    if key not in _CACHE:
        _CACHE.clear()
        sched, dinv, r_of, idx_blobs, dst_blobs, dinv_tiles = _preprocess(edge_index)
        nc = _build(sched)
        _CACHE[key] = (sched, dinv, r_of, idx_blobs, dst_blobs, dinv_tiles, nc)
    sched, dinv, r_of, idx_blobs, dst_blobs, dinv_tiles, nc = _CACHE[key]

    xt = np.empty((N, F_IN), np.float32)
    xt[r_of] = x * dinv[:, None]
    iota = np.tile(np.arange(DTILE, dtype=np.float32), (128, 1))

    Ws = [np.asarray(W1, np.float32)] + \
         [np.asarray(W2[i], np.float32) for i in range(L)] + \
         [np.asarray(W3, np.float32)]
    bs = [np.asarray(b1, np.float32)] + \
         [np.asarray(b2[i], np.float32) for i in range(L)] + \
         [np.asarray(b3, np.float32)]

    in_maps = []
    for c in range(NCORES):
        m = dict(xt=xt, idx16=idx_blobs[c], dstloc=dst_blobs[c],
                 iota=iota, dinvt=dinv_tiles[c])
        for l in range(L + 2):
            m[f"w{l}"] = Ws[l]
            m[f"bias{l}"] = np.tile(bs[l].reshape(1, -1), (128, 1)).astype(np.float32)
        in_maps.append(m)

    res = bass_utils.run_bass_kernel_spmd(nc, in_maps,
                                          core_ids=list(range(NCORES)))
    out = np.concatenate([res.results[c]["out"] for c in range(NCORES)], axis=0)
    return out
